# revision 1
# baseline (speedup 1.0000x reference)
"""Causal RoPE self-attention, distributed over 8 TRN2 NeuronCores.

Sharding: batch (2) x head-groups (4 heads each) -> 8 cores.
Each core computes, for its (batch b, head-group hg):
    q/k/v projections for its 4 heads (tensor-parallel column split),
    RoPE, causal attention, and the row-parallel slice of the output
    projection, producing a partial output partialT = WoS^T @ attnT
    of shape [E, S].  The host sums the 4 partials per batch and adds bo.

On-device layout notes:
  - activations live transposed: qT/kT are [head-dim, seq] so the
    score matmul sT[k, q] = K Q^T contracts over d on partitions (the
    two heads of a pair row-tile the PE array at K=64 each), and the
    softmax denominator comes from an extra all-ones column in V.
  - x, all weights, qT/kT, exp'd scores, and V are bf16 (full-rate
    TensorEngine, FWL weight loads, half DMA); every accumulation is
    fp32 in PSUM, and the softmax/normalization math is fp32.
  - causal masking zeroes the exp'd diagonal blocks with a gpsimd
    affine_select, off the DVE/PSUM critical path; exp(s/8) is safe
    unnormalized because |s/8| <~ 5 for this distribution.
  - input DMAs are interleaved per e-chunk (wk/wq/wv p-slices + the
    first S-half of x) so projection matmuls start as soon as the
    first chunks land instead of waiting for the full 7MB prefix.
  - attention PSUM accumulators are evicted to SBUF immediately after
    the last key-block so the next phase's AV matmuls never wait on
    the softmax-normalization chain; the denominator reciprocal is
    broadcast across partitions with gpsimd partition_broadcast
    (no DRAM round-trip).
  - the output projection is split into p0/p1 half-units staged
    through SBUF, so each half only needs one head-pair's attention
    output; partial outputs leave as bf16 (host sums in fp32).
"""

import ml_dtypes
import numpy as np

import concourse.tile as tile
from concourse import bacc, mybir
from concourse.bass_utils import run_bass_kernel_spmd

F32 = mybir.dt.float32
BF16 = mybir.dt.bfloat16
AF = mybir.ActivationFunctionType

B, S, E = 2, 2048, 1024
H, D = 16, 64
HPG = 4                # heads per core
DH = HPG * D           # 256 head-dims per core
NE = E // 128          # 8 e-chunks
NST = S // 128         # 16 s-tiles / key blocks
NSL = S // 512         # 4 q-slices
ROPE_BASE = 10000.0

_SWAP_MASK = [i ^ 1 for i in range(32)]


def build_nc():
    """Build + compile the per-core Bass graph (same graph on all 8 cores)."""
    nc = bacc.Bacc("TRN2", target_bir_lowering=False, debug=False, num_devices=8)

    def din(name, shape, dt=F32):
        return nc.dram_tensor(name, shape, dt, kind="ExternalInput").ap()

    xT = din("xT", [E, S], BF16)
    wqT = din("wqT", [E, DH], BF16)
    wkT = din("wkT", [E, DH], BF16)
    wvT = din("wvT", [E, DH], BF16)
    woST = din("woST", [DH, E], BF16)
    bq2 = din("bq2", [128, 2])
    bk2 = din("bk2", [128, 2])
    bvbc = din("bvbc", [128, DH])
    cos2 = din("cos2", [128, S], BF16)      # cosT duplicated on both halves
    sin2 = din("sin2", [128, S], BF16)      # signed sinT duplicated on both halves
    out = nc.dram_tensor("out", [E, S], BF16, kind="ExternalOutput").ap()

    xT_r = xT.rearrange("(n p) s -> n p s", p=128)
    wq_r = wqT.rearrange("(n p) d -> n p d", p=128)
    wk_r = wkT.rearrange("(n p) d -> n p d", p=128)
    wv_r = wvT.rearrange("(n p) d -> n p d", p=128)
    wo_r = woST.rearrange("(n p) e -> n p e", p=128)
    out_r = out.rearrange("(n p) s -> n p s", p=128)

    with tile.TileContext(nc) as tc, nc.allow_low_precision(
            reason="bf16 matmul operands; fp32 PSUM accumulation throughout"):
        _emit(tc, nc, dict(
            xT_r=xT_r, wq_r=wq_r, wk_r=wk_r, wv_r=wv_r, wo_r=wo_r, out_r=out_r,
            bq2=bq2, bk2=bk2, bvbc=bvbc, cos2=cos2, sin2=sin2,
        ))
    nc.compile()
    return nc


def _emit(tc, nc, d):
    from contextlib import ExitStack
    ctx = ExitStack()
    with ctx:
        consts = ctx.enter_context(tc.tile_pool(name="consts", bufs=1))
        px = ctx.enter_context(tc.tile_pool(name="px", bufs=16))
        pwq = ctx.enter_context(tc.tile_pool(name="pwq", bufs=16))
        pwk = ctx.enter_context(tc.tile_pool(name="pwk", bufs=16))
        pwv = ctx.enter_context(tc.tile_pool(name="pwv", bufs=8))
        pwo = ctx.enter_context(tc.tile_pool(name="pwo", bufs=2))
        pqt = ctx.enter_context(tc.tile_pool(name="pqt", bufs=4))
        pkt = ctx.enter_context(tc.tile_pool(name="pkt", bufs=4))
        pv = ctx.enter_context(tc.tile_pool(name="pv", bufs=16))
        pat = ctx.enter_context(tc.tile_pool(name="pat", bufs=8))
        ptmp = ctx.enter_context(tc.tile_pool(name="ptmp", bufs=6))
        pvf_ = ctx.enter_context(tc.tile_pool(name="pvf", bufs=3))
        pbc = ctx.enter_context(tc.tile_pool(name="pbc", bufs=4))
        pstg = ctx.enter_context(tc.tile_pool(name="pstg", bufs=8))
        pe_ = ctx.enter_context(tc.tile_pool(name="pe", bufs=10))
        prec = ctx.enter_context(tc.tile_pool(name="prec", bufs=4))
        psc = ctx.enter_context(tc.tile_pool(name="psc", bufs=2, space="PSUM"))
        ppv = ctx.enter_context(tc.tile_pool(name="ppv", bufs=2, space="PSUM"))
        pbg = ctx.enter_context(tc.tile_pool(name="pbg", bufs=2, space="PSUM"))

        # ---- input DMAs, interleaved per e-chunk in first-use order:
        # k/q/v weight p-slices + the first S-half of x land together so
        # the k00/q00 accumulation can start on chunk 0 immediately.
        wq_sb, wk_sb, wv_sb, wo_sb = {}, {}, {}, []
        xt_sb = {}

        def wslice(pool, src, e, p, store, tag):
            t = pool.tile([128, 128], BF16, tag=tag)
            nc.sync.dma_start(t, src[e][:, p * 128:(p + 1) * 128])
            store[(e, p)] = t

        for e in range(NE):
            wslice(pwk, d["wk_r"], e, 0, wk_sb, "wk")
            wslice(pwq, d["wq_r"], e, 0, wq_sb, "wq")
            t = pwv.tile([128, DH], BF16, tag="wv")
            nc.sync.dma_start(t, d["wv_r"][e])
            wv_sb[e] = t
            t = px.tile([128, 1024], BF16, tag="xt")
            nc.sync.dma_start(t, d["xT_r"][e][:, 0:1024])
            xt_sb[(e, 0)] = t
            if e == 4:
                cos2_sb = consts.tile([128, S], BF16)
                nc.sync.dma_start(cos2_sb, d["cos2"])
                sin2_sb = consts.tile([128, S], BF16)
                nc.sync.dma_start(sin2_sb, d["sin2"])
                bq2_sb = consts.tile([128, 2], F32)
                nc.sync.dma_start(bq2_sb, d["bq2"])
                bk2_sb = consts.tile([128, 2], F32)
                nc.sync.dma_start(bk2_sb, d["bk2"])
        bvbc_sb = consts.tile([128, DH], F32)
        nc.sync.dma_start(bvbc_sb, d["bvbc"])
        for e in range(NE):
            wslice(pwk, d["wk_r"], e, 1, wk_sb, "wk")
            wslice(pwq, d["wq_r"], e, 1, wq_sb, "wq")
            t = px.tile([128, 1024], BF16, tag="xt")
            nc.sync.dma_start(t, d["xT_r"][e][:, 1024:2048])
            xt_sb[(e, 1)] = t
        for p in range(2):
            t = pwo.tile([128, E], BF16, tag="wo")
            nc.sync.dma_start(t, d["wo_r"][p])
            wo_sb.append(t)

        def xs(e, scol, w=512):
            h, off = divmod(scol, 1024)
            return xt_sb[(e, h)][:, off:off + w]

        # ---- emission: a fine-grained interleave. The softmax exps on the
        # scalar engine pace the attention stream (~0.9us/key-block); all
        # other PE work (projections, V, output projection) is split into
        # small "background" units drip-fed between key-blocks so neither
        # engine starves and the PE clock stays unthrottled.
        qt_tiles, kt_tiles, at_tiles = {}, {}, {}
        v_sb = {}
        op_stage = {}

        def rope_tail(ps_pair, bias_sb, dst_pool, dst_tag, tiles, p, sp):
            cols = slice(sp * 1024, (sp + 1) * 1024)
            tq = ptmp.tile([128, 1024], BF16, tag="tmpb")
            tqr = tq.rearrange("p (a b) -> p a b", b=512)
            nc.vector.tensor_scalar_add(tqr[:, 0, :], ps_pair[0], bias_sb[:, p:p + 1])
            nc.vector.tensor_scalar_add(tqr[:, 1, :], ps_pair[1], bias_sb[:, p:p + 1])
            tsh = ptmp.tile([128, 1024], BF16, tag="tmpb")
            nc.vector.stream_shuffle(tsh, tq, _SWAP_MASK)
            nc.vector.tensor_mul(tsh, tsh, sin2_sb[:, cols])
            nc.vector.tensor_mul(tq, tq, cos2_sb[:, cols])
            qt = dst_pool.tile([128, 1024], BF16, tag=dst_tag)
            nc.vector.tensor_add(qt, tq, tsh)
            tiles[(p, sp)] = qt

        def emit_qk_unit(w_sb, bias_sb, dst_pool, dst_tag, tiles, p, sp):
            """Yields one background unit per e-chunk, then a RoPE tail."""
            ps0 = pbg.tile([128, 512], F32, tag="bg")
            ps1 = pbg.tile([128, 512], F32, tag="bg")
            halves = (ps0, ps1)
            for e in range(NE):
                def unit(e=e):
                    for half in range(2):
                        scol = (sp * 2 + half) * 512
                        nc.tensor.matmul(
                            halves[half],
                            w_sb[(e, p)],
                            xs(e, scol),
                            start=(e == 0), stop=(e == NE - 1),
                        )
                yield 0.45, unit
            yield 0.1, lambda: rope_tail(
                halves, bias_sb, dst_pool, dst_tag, tiles, p, sp)

        def emit_v_unit(st):
            def unit():
                psv = pbg.tile([128, DH], F32, tag="bg")
                for e in range(NE):
                    nc.tensor.matmul(
                        psv,
                        xs(e, st * 128, 128),
                        wv_sb[e],
                        start=(e == 0), stop=(e == NE - 1),
                    )
                vt = pv.tile([128, HPG, 65], BF16, tag="v")
                nc.vector.memset(vt[:, :, 64:65], 1.0)
                nc.vector.tensor_add(
                    vt[:, :, 0:64],
                    psv.rearrange("p (h dd) -> p h dd", dd=64),
                    bvbc_sb.rearrange("p (h dd) -> p h dd", dd=64),
                )
                v_sb[st] = vt
            yield 1.1, unit

        def emit_op_unit(j):
            """Fused output-projection unit: both head-pairs accumulate in
            PSUM, one bf16 staging copy, one DMA."""
            for et_i in range(NE):
                def unit(et_i=et_i):
                    pso = pbg.tile([128, 512], F32, tag="bg")
                    for p in range(2):
                        nc.tensor.matmul(
                            pso,
                            wo_sb[p][:, et_i * 128:(et_i + 1) * 128],
                            at_tiles[(p, j)],
                            start=(p == 0), stop=(p == 1),
                        )
                    ob = pstg.tile([128, 512], BF16, tag="ob")
                    nc.vector.tensor_copy(ob, pso)
                    nc.sync.dma_start(
                        d["out_r"][et_i][:, j * 512:(j + 1) * 512], ob)
                yield 0.7, unit

        def emit_op_p0(j):
            """First head-pair's half of the output projection for q-slice j;
            accumulates into an SBUF stage so it can run as soon as at(0,j)
            exists, one attention phase before at(1,j). Used for the final
            q-slice only, to shorten the tail."""
            for et_i in range(NE):
                def unit(et_i=et_i):
                    pso = pbg.tile([128, 512], F32, tag="bg")
                    nc.tensor.matmul(
                        pso,
                        wo_sb[0][:, et_i * 128:(et_i + 1) * 128],
                        at_tiles[(0, j)],
                        start=True, stop=True,
                    )
                    stg = pstg.tile([128, 512], F32, tag="stg")
                    nc.vector.tensor_copy(stg, pso)
                    op_stage[(j, et_i)] = stg
                yield 0.4, unit

        def emit_op_p1(j):
            for et_i in range(NE):
                def unit(et_i=et_i):
                    pso = pbg.tile([128, 512], F32, tag="bg")
                    nc.tensor.matmul(
                        pso,
                        wo_sb[1][:, et_i * 128:(et_i + 1) * 128],
                        at_tiles[(1, j)],
                        start=True, stop=True,
                    )
                    ob = pstg.tile([128, 512], BF16, tag="ob")
                    nc.vector.tensor_add(ob, op_stage[(j, et_i)], pso)
                    nc.sync.dma_start(
                        d["out_r"][et_i][:, j * 512:(j + 1) * 512], ob)
                yield 0.45, unit

        # background unit queue + driver
        bg_units = []
        bg_pos = [0]

        def bg_add(gen):
            bg_units.extend(gen)
            return len(bg_units)

        def bg_flush_until(idx):
            while bg_pos[0] < idx:
                bg_units[bg_pos[0]][1]()
                bg_pos[0] += 1

        def bg_take(budget):
            while budget > 0 and bg_pos[0] < len(bg_units):
                cost, fn = bg_units[bg_pos[0]]
                fn()
                bg_pos[0] += 1
                budget -= cost

        def emit_attn(p, j, take=0.45):
            pvA = ppv.tile([128, 512], F32, tag="ppv")
            pvB = ppv.tile([128, 512], F32, tag="ppv")
            nkb = 4 * j + 4
            for kb in range(nkb):
                m = kb - 4 * j
                c0 = 128 * m if m > 0 else 0
                kt = kt_tiles[(p, kb // 8)]
                kcols = slice((kb % 8) * 128, (kb % 8) * 128 + 128)
                qt = qt_tiles[(p, j // 2)]
                qcols = slice((j % 2) * 512 + c0, (j % 2) * 512 + 512)
                sc = psc.tile([128, 2, 512], F32, tag="sc")
                nc.tensor.matmul(
                    sc[:, 0, c0:512],
                    kt[0:64, kcols],
                    qt[0:64, qcols],
                    start=True, stop=True, tile_position=(0, 0),
                )
                nc.tensor.matmul(
                    sc[:, 1, c0:512],
                    kt[64:128, kcols],
                    qt[64:128, qcols],
                    start=True, stop=True, tile_position=(64, 0),
                )
                et = pe_.tile([128, 2, 512], BF16, tag="e")
                nc.scalar.activation(
                    et[:, :, c0:512], sc[:, :, c0:512], AF.Exp, scale=0.125)
                if m >= 0:
                    nc.gpsimd.affine_select(
                        out=et[:, :, c0:c0 + 128],
                        in_=et[:, :, c0:c0 + 128],
                        compare_op=mybir.AluOpType.is_ge,
                        fill=0.0,
                        base=0,
                        pattern=[[0, 2], [1, 128]],
                        channel_multiplier=-1,
                    )
                hA, hB = 2 * p, 2 * p + 1
                nc.tensor.matmul(
                    pvA[0:65, c0:512], v_sb[kb][:, hA, :], et[:, 0, c0:512],
                    start=(kb == 0), stop=(kb == nkb - 1),
                )
                nc.tensor.matmul(
                    pvB[0:65, c0:512], v_sb[kb][:, hB, :], et[:, 1, c0:512],
                    start=(kb == 0), stop=(kb == nkb - 1),
                )
                bg_take(take)
            # Evict the PSUM accumulators to SBUF right away: frees both ppv
            # banks for the next phase's AV matmuls, and the normalization
            # chain below runs entirely out of SBUF.
            pvf = pvf_.tile([128, 1024], F32, tag="pvf")
            nc.vector.tensor_copy(pvf[0:65, 0:512], pvA[0:65, :])
            nc.vector.tensor_copy(pvf[0:65, 512:1024], pvB[0:65, :])
            # den must be a base-0 AP: custom-DVE ops (reciprocal) ignore the
            # input's base partition, so slice pvf[64:65] via a copy first.
            den = prec.tile([1, 1024], F32, tag="den")
            nc.vector.tensor_copy(den, pvf[64:65, :])
            rec = prec.tile([1, 1024], F32, tag="rec")
            nc.vector.reciprocal_approx_fast(rec, den)
            bcsA = pbc.tile([64, 512], F32, tag="bc")
            bcsB = pbc.tile([64, 512], F32, tag="bc")
            nc.gpsimd.partition_broadcast(bcsA, rec[:, 0:512])
            nc.gpsimd.partition_broadcast(bcsB, rec[:, 512:1024])
            at = pat.tile([128, 512], BF16, tag="at")
            nc.vector.tensor_mul(at[0:64], pvf[0:64, 0:512], bcsA)
            nc.vector.tensor_mul(at[64:128], pvf[0:64, 512:1024], bcsB)
            at_tiles[(p, j)] = at

        def emit_qk_psc(w_sb, bias_sb, dst_pool, dst_tag, tiles, p, sp):
            """k00 accumulates in the scores pool (idle before attention) and
            interleaves with q00 per e-chunk to track the input DMA stream."""
            ps = psc.tile([128, 2, 512], F32, tag="sc")
            ps0 = pbg.tile([128, 512], F32, tag="bg")
            ps1 = pbg.tile([128, 512], F32, tag="bg")
            for e in range(NE):
                for half in range(2):
                    nc.tensor.matmul(
                        ps[:, half, :],
                        w_sb[(e, p)],
                        xs(e, (sp * 2 + half) * 512),
                        start=(e == 0), stop=(e == NE - 1),
                    )
                nc.tensor.matmul(
                    ps0, wq_sb[(e, p)], xs(e, sp * 1024),
                    start=(e == 0), stop=(e == NE - 1))
                nc.tensor.matmul(
                    ps1, wq_sb[(e, p)], xs(e, sp * 1024 + 512),
                    start=(e == 0), stop=(e == NE - 1))
            tqk = ptmp.tile([128, 1024], BF16, tag="tmpb")
            tqkr = tqk.rearrange("p (a b) -> p a b", b=512)
            nc.vector.tensor_scalar_add(tqkr, ps, bias_sb[:, p:p + 1])
            cols = slice(sp * 1024, (sp + 1) * 1024)
            tsh = ptmp.tile([128, 1024], BF16, tag="tmpb")
            nc.vector.stream_shuffle(tsh, tqk, _SWAP_MASK)
            nc.vector.tensor_mul(tsh, tsh, sin2_sb[:, cols])
            nc.vector.tensor_mul(tqk, tqk, cos2_sb[:, cols])
            kt = dst_pool.tile([128, 1024], BF16, tag=dst_tag)
            nc.vector.tensor_add(kt, tqk, tsh)
            tiles[(p, sp)] = kt
            rope_tail((ps0, ps1), bq2_sb, pqt, "qt", qt_tiles, p, sp)

        # ---- schedule ----
        # pre-phase: k00+q00 interleaved per e-chunk (tracks DMA arrival),
        # then v0-3 so attention phase (0,0) can start the exp stream early.
        emit_qk_psc(wk_sb, bk2_sb, pkt, "kt", kt_tiles, 0, 0)
        for st in range(0, 4):
            for cost, fn in emit_v_unit(st):
                fn()

        # background queue, ordered so prerequisites precede each phase
        bg_add(emit_v_unit(4)); bg_add(emit_v_unit(5))
        bg_add(emit_v_unit(6)); i_v7 = bg_add(emit_v_unit(7))
        bg_add(emit_qk_unit(wk_sb, bk2_sb, pkt, "kt", kt_tiles, 1, 0))
        i_p10 = bg_add(emit_qk_unit(wq_sb, bq2_sb, pqt, "qt", qt_tiles, 1, 0))
        bg_add(emit_v_unit(8)); bg_add(emit_v_unit(9))
        bg_add(emit_v_unit(10)); i_v11 = bg_add(emit_v_unit(11))
        bg_add(emit_qk_unit(wk_sb, bk2_sb, pkt, "kt", kt_tiles, 0, 1))
        i_p01 = bg_add(emit_qk_unit(wq_sb, bq2_sb, pqt, "qt", qt_tiles, 0, 1))
        bg_add(emit_qk_unit(wk_sb, bk2_sb, pkt, "kt", kt_tiles, 1, 1))
        i_p11 = bg_add(emit_qk_unit(wq_sb, bq2_sb, pqt, "qt", qt_tiles, 1, 1))
        bg_add(emit_v_unit(12)); bg_add(emit_v_unit(13))
        bg_add(emit_v_unit(14)); i_v15 = bg_add(emit_v_unit(15))

        # op halves are queued strictly alternating p0(j), p1(j), p0(j+1), ...
        # so the 8-deep stage pool ring never creates a dependency on a
        # later instruction (each p0(j+1) unit reuses the stage buffer its
        # matching p1(j) unit - queued earlier - has already read).
        emit_attn(0, 0)
        bg_flush_until(i_v7)
        emit_attn(0, 1)
        bg_flush_until(i_p10)
        emit_attn(1, 0)
        bg_add(emit_op_unit(0))
        emit_attn(1, 1)
        bg_add(emit_op_unit(1))
        bg_flush_until(i_p01)
        emit_attn(0, 2)
        bg_flush_until(i_p11)
        emit_attn(1, 2)
        bg_add(emit_op_unit(2))
        bg_flush_until(i_v15)
        emit_attn(0, 3, take=0.55)
        bg_add(emit_op_p0(3))
        emit_attn(1, 3, take=0.55)
        bg_flush_until(len(bg_units))
        for cost, fn in emit_op_p1(3):
            fn()


def make_host_inputs(x, Wq, bq, Wk, bk, Wv, bv, Wo, bo):
    """Shard + pre-transpose inputs per core. Returns (in_maps, bo)."""
    x = np.asarray(x, np.float32)
    Wq, Wk, Wv, Wo = (np.asarray(w, np.float32) for w in (Wq, Wk, Wv, Wo))
    bq, bk, bv, bo = (np.asarray(b_, np.float32) for b_ in (bq, bk, bv, bo))

    # RoPE tables
    half = D // 2
    inv_freq = 1.0 / (ROPE_BASE ** (np.arange(half, dtype=np.float64) / half))
    pos = np.arange(S, dtype=np.float64)
    sinus = pos[:, None] * inv_freq[None, :]           # [S, 32]
    sin_full = np.repeat(np.sin(sinus), 2, axis=1)     # [S, 64] interleave-dup
    cos_full = np.repeat(np.cos(sinus), 2, axis=1)
    sgn = np.where(np.arange(D) % 2 == 0, -1.0, 1.0)
    cos2 = np.tile(cos_full.T, (2, 1)).astype(ml_dtypes.bfloat16)
    sin2 = np.tile((sin_full * sgn[None, :]).T, (2, 1)).astype(ml_dtypes.bfloat16)

    xT = [np.ascontiguousarray(x[b_].T) for b_ in range(B)]
    in_maps = []
    for c in range(8):
        b_, hg = c // 4, c % 4
        rows = slice(DH * hg, DH * hg + DH)
        bf = ml_dtypes.bfloat16
        in_maps.append({
            "xT": xT[b_].astype(bf),
            "wqT": np.ascontiguousarray(Wq[rows].T).astype(bf),
            "wkT": np.ascontiguousarray(Wk[rows].T).astype(bf),
            "wvT": np.ascontiguousarray(Wv[rows].T).astype(bf),
            "woST": np.ascontiguousarray(Wo[:, rows].T).astype(bf),
            "bq2": np.ascontiguousarray(bq[rows].reshape(2, 128).T),
            "bk2": np.ascontiguousarray(bk[rows].reshape(2, 128).T),
            "bvbc": np.tile(bv[rows][None, :], (128, 1)).astype(np.float32),
            "cos2": cos2,
            "sin2": sin2,
        })
    return in_maps, bo


_NC_CACHE = {}


def get_nc():
    if "nc" not in _NC_CACHE:
        _NC_CACHE["nc"] = build_nc()
    return _NC_CACHE["nc"]


def kernel(**inputs):
    in_maps, bo = make_host_inputs(**inputs)
    nc = get_nc()
    res = run_bass_kernel_spmd(nc, in_maps, core_ids=list(range(8)))
    out = np.zeros((B, S, E), np.float32)
    for c in range(8):
        out[c // 4] += np.asarray(res.results[c]["out"], np.float32).T
    out += bo[None, None, :]
    return out

